# revision 1
# baseline (speedup 1.0000x reference)
"""Trainium2 Bass kernel for nn_Axial_PFCU_Continuous (dense_cnn).

Math (per sample, C=96, H=W=128), folded host-side:
  m+l   = 2x + sum_d chA[d] (.) shiftH(x,d) + sum_d cwA[d] (.) shiftW(x,d)
  fused = Wf~ @ (m+l) + bf                  (BN folded)
  anchor= 3-tap dw convs of x along H and W (BN folded) + bias
  z     = fused + anchor ; pre = PReLU(z, a)
  coord attention: spatial means of pre -> tiny matmuls -> sigmoid gates;
  out   = pre * ah(c,h) * aw(c,w)

Sharding: pure data-parallel, 1 of 8 batch samples per NeuronCore.

Per core (layout: C=96 partitions, H*W=16384 free; hot path bf16):
  PE  : z-chunk matmuls — Wf~@s, ident@anchor-acc, 9 shift terms as
        (Wf~ diag(c_d)) / diag matmuls on shifted rhs windows (LDWEIGHTS
        paired across the two chunks of each block), xw-mean accumulation,
        coord-att matmuls, HAM prologue warmers
  DVE : tap-scales (tensor_scalar, 4x mode) for the 7 chain terms, two
        chain adds, per-block xh fold(2x)+reduce, finals TT1 (x aw)
  GPS : five chain accumulate-adds per block, finals TT2 (x ah)
  ACT : per-block s/anchor inits (Identity, per-channel scale+bias),
        PSUM evict fused with PReLU (bf16 out), coord-att PReLU/Sigmoids

Pipeline: 18 h-blocks (4-row halves at both edges for faster fill/drain,
8-row interior), two 512-col PSUM chunk groups per block.
"""
import sys
import math

sys.path.insert(0, '/opt/trn_rl_repo')

import numpy as np
import ml_dtypes
from contextlib import ExitStack

import concourse.bass as bass
import concourse.bacc as bacc
from concourse import mybir, tile
from concourse.bass_utils import run_bass_kernel_spmd

f32 = mybir.dt.float32
bf16 = mybir.dt.bfloat16
ALU = mybir.AluOpType
AF = mybir.ActivationFunctionType

B, C, H, W = 8, 96, 128, 128
HW = H * W
EPS = 1e-5
N_CORES = 8

NBLK = 16           # h-blocks per sample (DVE term ops + DMA chunking)
BH = H // NBLK      # 16 h-rows per block
NCHUNK = 2          # 512-col matmul chunks per block (one PSUM bank)
CW = BH * W // NCHUNK  # 512
CH = CW // W        # 4 h-rows per chunk

# shift terms routed to PE as accumulating matmuls (rest go to DVE STT)
PE_A_H = (-16, -8, -4, 8, 16)
PE_A_W = (-16, 16)
PE_B_W = True       # +-1 W-shift taps as PE diag passes

_GRAPH_CACHE = {}


# ----------------------------------------------------------------- host folds
def _taps(w_taps, r):
    """offset -> (C,) coefficient for the integer-shift decomposition."""
    r = max(float(r), 1.0)
    K = w_taps.shape[1]
    d2w = {}
    for i in range(K):
        s = (i - K // 2) * r
        f = math.floor(s)
        frac = s - f
        for d, wt in ((int(f), 1.0 - frac), (int(f) + 1, frac)):
            if wt != 0.0:
                if d not in d2w:
                    d2w[d] = np.zeros(C, np.float64)
                d2w[d] = d2w[d] + wt * np.asarray(w_taps[:, i], np.float64)
    return {d: w for d, w in d2w.items() if abs(d) < H}


def _merge(a, b):
    out = dict(a)
    for d, w in b.items():
        out[d] = out.get(d, np.zeros(C, np.float64)) + w
    return out


class _Pack:
    def __init__(self):
        self.cols = {}
        self.parts = []
        self.pos = 0

    def put(self, name, arr):
        arr = np.asarray(arr, np.float64)
        if arr.ndim == 1:
            arr = arr[:, None]
        pad = np.zeros((C, arr.shape[1]), np.float64)
        pad[:arr.shape[0], :] = arr
        self.cols[name] = (self.pos, arr.shape[1])
        self.parts.append(pad)
        self.pos += arr.shape[1]

    def done(self, dt):
        return np.concatenate(self.parts, axis=1).astype(dt)


def _fold(inp):
    g = lambda k: np.asarray(inp[k], np.float64)
    hA = _merge(_taps(g('wh_m'), float(np.asarray(inp['r_m']))),
                _taps(g('wh_l'), float(np.asarray(inp['r_l']))))
    wA = _merge(_taps(g('ww_m'), float(np.asarray(inp['r_m']))),
                _taps(g('ww_l'), float(np.asarray(inp['r_l']))))
    hA[0] = hA.get(0, np.zeros(C)) + 2.0    # identity terms of m+l
    wA.setdefault(0, np.zeros(C))

    pe_h = tuple(d for d in sorted(hA) if d != 0 and d in PE_A_H)
    pe_w = tuple(d for d in sorted(wA) if d != 0 and d in PE_A_W)
    dve_h = tuple(d for d in sorted(hA) if d != 0 and d not in pe_h)
    dve_w = tuple(d for d in sorted(wA) if d != 0 and d not in pe_w)

    sf = g('bnf_g') / np.sqrt(g('bnf_v') + EPS)
    wfuse_t = (g('w_fuse') * sf[:, None]).T.copy()      # (Cin, Cout) lhsT
    bf = g('bnf_b') - g('bnf_m') * sf

    ds = g('dg_g') / np.sqrt(g('dg_v') + EPS)
    db = g('dg_b') - g('dg_m') * ds
    dg_wh, dg_ww = g('dg_wh'), g('dg_ww')
    ehm1, eh0, ehp1 = ds * dg_wh[:, 0], ds * (dg_wh[:, 1] + 1.0), ds * dg_wh[:, 2]
    ewm1, ew0, ewp1 = ds * dg_ww[:, 0], ds * dg_ww[:, 1], ds * dg_ww[:, 2]

    cs = g('ca_g') / np.sqrt(g('ca_v') + EPS)
    cb = g('ca_b') - g('ca_m') * cs

    # f32 consts (STT scalars, CA chain, biases)
    pkf = _Pack()
    pkf.put('cA0', hA[0] + wA[0])
    if dve_h:
        pkf.put('chA', np.stack([hA[d] for d in dve_h], 1))
    if dve_w:
        pkf.put('cwA', np.stack([wA[d] for d in dve_w], 1))
    pkf.put('cB0', eh0 + ew0)
    pkf.put('bz', bf + db)
    pkf.put('ehm1', ehm1); pkf.put('ehp1', ehp1)
    if not PE_B_W:
        pkf.put('ewm1', ewm1); pkf.put('ewp1', ewp1)
    pkf.put('act_a', g('act_a'))
    pkf.put('zero', np.zeros(C))
    pkf.put('caw1_t', (g('ca_w1') / float(W)).T)   # (C, 8); 1/W mean fold
    pkf.put('cas', cs); pkf.put('cab', cb); pkf.put('caa', g('ca_a'))
    pkf.put('cawh_t', g('ca_wh').T)                # (8, C)
    pkf.put('caww_t', g('ca_ww').T)
    consts = pkf.done(np.float32)

    # bf16 consts (matmul stationaries)
    pkb = _Pack()
    pkb.put('wfuse_t', wfuse_t)
    pkb.put('ident', np.eye(C))
    for d in pe_h:
        pkb.put(f'Ah{d}', wfuse_t * hA[d][:, None])
    for d in pe_w:
        pkb.put(f'Aw{d}', wfuse_t * wA[d][:, None])
    if PE_B_W:
        pkb.put('Dwm1', np.diag(ewm1))
        pkb.put('Dwp1', np.diag(ewp1))
    constb = pkb.done(ml_dtypes.bfloat16)

    key = (dve_h, dve_w, pe_h, pe_w, consts.shape[1], constb.shape[1])
    return consts, pkf.cols, constb, pkb.cols, dve_h, dve_w, pe_h, pe_w, key


# -------------------------------------------------------------- graph builder
def _build(dve_h, dve_w, pe_h, pe_w, colf, colb, ckf, ckb):
    nc = bacc.Bacc()
    x_p = nc.declare_dram_parameter("x", (C, HW), bf16, isOutput=False)
    cf_p = nc.declare_dram_parameter("consts", (C, ckf), f32, isOutput=False)
    cb_p = nc.declare_dram_parameter("constb", (C, ckb), bf16, isOutput=False)
    o_p = nc.declare_dram_parameter("out", (C, HW), bf16, isOutput=True)

    with tile.TileContext(nc) as tc, ExitStack() as ctx:
        big = ctx.enter_context(tc.tile_pool(name="big", bufs=1))
        sp = ctx.enter_context(tc.tile_pool(name="sp", bufs=8))
        op = ctx.enter_context(tc.tile_pool(name="op", bufs=6))
        tp = ctx.enter_context(tc.tile_pool(name="tp", bufs=12))
        psq = ctx.enter_context(tc.tile_pool(name="psq", bufs=4, space="PSUM"))
        psa = ctx.enter_context(tc.tile_pool(name="psa", bufs=1, space="PSUM"))
        pss = ctx.enter_context(tc.tile_pool(name="pss", bufs=2, space="PSUM"))

        cst = big.tile([C, ckf], f32, tag="cst")
        nc.sync.dma_start(cst[:], cf_p[:])
        cbt = big.tile([C, ckb], bf16, tag="cbt")
        nc.sync.dma_start(cbt[:], cb_p[:])

        def cc(name, i=0):
            p0, n = colf[name]
            assert i < n
            return cst[:, p0 + i:p0 + i + 1]

        def cbr(name, rows=C):
            p0, n = colb[name]
            return cbt[0:rows, p0:p0 + n]

        def crf(name, rows=C):
            p0, n = colf[name]
            return cst[0:rows, p0:p0 + n]

        x_sb = big.tile([C, HW], bf16, tag="x")
        for j in range(NBLK):
            sl = slice(j * HW // NBLK, (j + 1) * HW // NBLK)
            nc.sync.dma_start(x_sb[:, sl], x_p[:, sl])
        x3 = x_sb[:].rearrange("p (h w) -> p h w", w=W)

        ac_sb = big.tile([C, HW], bf16, tag="ac")
        ac3 = ac_sb[:].rearrange("p (h w) -> p h w", w=W)

        zcol = cc('zero')
        # engine warmups (observe const tiles once; preload ACT tables)
        wrm = big.tile([C, 4], f32, tag="wrm")
        nc.scalar.activation(wrm[:, 0:1], zcol, AF.Prelu, bias=zcol, scale=1.0,
                             alpha=cc('act_a'))
        nc.scalar.activation(wrm[:, 3:4], zcol, AF.Sigmoid, bias=zcol, scale=1.0)
        nc.vector.tensor_copy(wrm[:, 1:2], zcol)
        nc.gpsimd.tensor_copy(wrm[:, 2:3], zcol)
        psw = pss.tile([C, 1], f32, tag="small")
        nc.tensor.matmul(psw[:], cbr('ident'), cbr('ident')[:, 0:1],
                         start=True, stop=True)
        # PE HAM warmers: fill the otherwise-idle prologue with matmul work
        # so the first real chunk matmuls run at the warm clock
        wck = min(512, (colb[list(colb)[-1]][0] + colb[list(colb)[-1]][1]))
        pwm = pss.tile([C, wck], f32, tag="small")
        for wi in range(6):
            nc.tensor.matmul(pwm[:], cbr('ident'), cbt[0:C, 0:wck],
                             start=(wi == 0), stop=(wi == 5))

        # running xw-mean accumulator (filled per chunk inside the loop)
        xwp = psa.tile([C, CH, W], f32, tag="xwp")
        yin = big.tile([C, 2 * H], f32, tag="yin")

        # per-block: DVE shift-MACs into s / anchor into ac, then PE chunks.
        # First/last blocks split in half: shorter chains fill the pipeline
        # faster at the start and drain it earlier at the end.
        bounds = ([(0, BH // 2), (BH // 2, BH),
                   (BH, BH + BH // 2), (BH + BH // 2, 2 * BH)]
                  + [(j * BH, (j + 1) * BH) for j in range(2, NBLK - 2)]
                  + [((NBLK - 2) * BH, H - BH - BH // 2),
                     (H - BH - BH // 2, H - BH),
                     (H - BH, H - BH // 2), (H - BH // 2, H)])
        gcnt = [0]
        for r0, r1 in bounds:
            bh = r1 - r0
            # anchor: init (with bias) + remaining taps
            nc.scalar.activation(ac_sb[:, r0 * W:r1 * W],
                                 x_sb[:, r0 * W:r1 * W], AF.Identity,
                                 bias=cc('bz'), scale=cc('cB0'))
            for coef, dh, kind in (('ehm1', -1, 'pool'), ('ehp1', 1, 'dve')):
                a = max(r0, -dh); b = min(r1, H - dh)
                dsl = ac3[:, a:b, :]
                ssl = x3[:, a + dh:b + dh, :]
                tmp = tp.tile([C, bh * W], bf16, tag="tmp")
                t3 = tmp[:].rearrange("p (h w) -> p h w", w=W)[:, 0:b - a, :]
                nc.vector.tensor_scalar(t3, ssl, cc(coef), None, ALU.mult)
                kadd = ('pool' if kind == 'pool' else kind)
                if kadd == 'dma':
                    nc.gpsimd.dma_start(dsl, t3, accum_op=ALU.add)
                elif kadd == 'pool':
                    nc.gpsimd.tensor_tensor(dsl, dsl, t3, op=ALU.add)
                else:
                    nc.vector.tensor_tensor(dsl, dsl, t3, op=ALU.add)
            if not PE_B_W:
                for coef, dw in (('ewm1', -1), ('ewp1', 1)):
                    wa = max(0, -dw); wb = min(W, W - dw)
                    dsl = ac3[:, r0:r1, wa:wb]
                    ssl = x3[:, r0:r1, wa + dw:wb + dw]
                    tmp = tp.tile([C, BH * W], bf16, tag="tmp")
                    t3 = tmp[:].rearrange("p (h w) -> p h w", w=W)[:, :, 0:wb - wa]
                    nc.vector.tensor_scalar(t3, ssl, cc(coef), None, ALU.mult)
                    nc.gpsimd.tensor_tensor(dsl, dsl, t3, op=ALU.add)

            s_t = sp.tile([C, bh * W], bf16, tag="s")
            s3 = s_t[:].rearrange("p (h w) -> p h w", w=W)

            # s init with the d=0 coefficient (4x mode)
            nc.scalar.activation(s_t[:], x_sb[:, r0 * W:r1 * W], AF.Identity,
                                 bias=zcol, scale=cc('cA0'))
            def adder(kind):
                if kind == 'pool':
                    return lambda dsl, t3: nc.gpsimd.tensor_tensor(
                        dsl, dsl, t3, op=ALU.add)
                if kind == 'dve':
                    return lambda dsl, t3: nc.vector.tensor_tensor(
                        dsl, dsl, t3, op=ALU.add)
                return lambda dsl, t3: nc.gpsimd.dma_start(
                    dsl, t3, accum_op=ALU.add)

            for i, d in enumerate(dve_h):
                a = max(r0, -d); b = min(r1, H - d)
                if b <= a:
                    continue
                dsl = s3[:, a - r0:b - r0, :]
                ssl = x3[:, a + d:b + d, :]
                tmp = tp.tile([C, bh * W], bf16, tag="tmp")
                t3 = tmp[:].rearrange("p (h w) -> p h w", w=W)[:, 0:b - a, :]
                nc.vector.tensor_scalar(t3, ssl, cc('chA', i), None, ALU.mult)
                adder('dve')(dsl, t3)
            W_ADD = {}
            for i, d in enumerate(dve_w):
                wa = max(0, -d); wb = min(W, W - d)
                dsl = s3[:, :, wa:wb]
                ssl = x3[:, r0:r1, wa + d:wb + d]
                tmp = tp.tile([C, bh * W], bf16, tag="tmp")
                t3 = tmp[:].rearrange("p (h w) -> p h w", w=W)[:, :, 0:wb - wa]
                nc.vector.tensor_scalar(t3, ssl, cc('cwA', i), None, ALU.mult)
                adder(W_ADD.get(d, 'pool'))(dsl, t3)

            # matmul chunks: psum = Wf~@s + ident@ac + PE shift terms.
            # Both chunks of the block are emitted interleaved per
            # stationary so each LDWEIGHTS serves two matmuls.
            order = (['ident'] + [f'Ah{d}' for d in pe_h]
                     + [f'Aw{d}' for d in pe_w]
                     + (['Dwm1', 'Dwp1'] if PE_B_W else []) + ['wfuse_t'])
            nch = bh * W // CW
            cmms = []
            pks = []
            for k in range(nch):
                cr0 = r0 + k * CH                  # first global h-row
                col0 = cr0 * W
                pk_t = psq.tile([C, CH, W], f32, tag="pk")
                pks.append((pk_t, cr0, col0))
                mm = {}
                mm['ident'] = (cbr('ident'), ac_sb[:, col0:col0 + CW], None)
                for d in pe_h:
                    a = max(cr0, -d); b = min(cr0 + CH, H - d)
                    if b <= a:
                        continue
                    mm[f'Ah{d}'] = (cbr(f'Ah{d}'), x3[:, a + d:b + d, :],
                                    pk_t[:, a - cr0:b - cr0, :])
                for d in pe_w:
                    wa = max(0, -d); wb = min(W, W - d)
                    mm[f'Aw{d}'] = (cbr(f'Aw{d}'),
                                    x3[:, cr0:cr0 + CH, wa + d:wb + d],
                                    pk_t[:, :, wa:wb])
                if PE_B_W:
                    for nm, d in (('Dwm1', -1), ('Dwp1', 1)):
                        wa = max(0, -d); wb = min(W, W - d)
                        mm[nm] = (cbr(nm),
                                  x3[:, cr0:cr0 + CH, wa + d:wb + d],
                                  pk_t[:, :, wa:wb])
                mm['wfuse_t'] = (cbr('wfuse_t'),
                                 s_t[:, k * CW:(k + 1) * CW], None)
                cmms.append(mm)
            for name in order:
                for k in range(nch):
                    if name not in cmms[k]:
                        continue
                    lhsT, rhs, out = cmms[k][name]
                    o = out if out is not None else pks[k][0][:]
                    nc.tensor.matmul(o, lhsT, rhs, start=(name == 'ident'),
                                     stop=(name == 'wfuse_t'))
            for k in range(nch):
                pk_t, cr0, col0 = pks[k]
                nc.scalar.activation(ac_sb[:, col0:col0 + CW], pk_t[:],
                                     AF.Prelu, bias=zcol, scale=1.0,
                                     alpha=cc('act_a'))
            for k in range(nch):
                pk_t, cr0, col0 = pks[k]
                nc.tensor.matmul(xwp[:], cbr('ident'),
                                 ac3[:, cr0:cr0 + CH, :],
                                 start=(gcnt[0] == 0),
                                 stop=(gcnt[0] == NBLK * NCHUNK - 1))
                gcnt[0] += 1

            # xh rows for this block: fold w in half at 2x, then 1x reduce.
            # The fold is off the block critical chain (consumed only at the
            # coord-att tail), so alternate it onto Pool for busy relief;
            # keep the last blocks on DVE (their fold IS tail-latency).
            fold = tp.tile([C, bh * (W // 2)], bf16, tag="fold")
            f3 = fold[:].rearrange("p (h w) -> p h w", w=W // 2)
            feng = nc.gpsimd if (r0 // BH) % 2 == 0 and r1 <= H - BH else nc.vector
            feng.tensor_tensor(f3[:], ac3[:, r0:r1, 0:W // 2],
                               ac3[:, r0:r1, W // 2:W], op=ALU.add)
            nc.vector.tensor_reduce(yin[:, r0:r1], f3[:],
                                    axis=mybir.AxisListType.X, op=ALU.add)

        nc.vector.tensor_reduce(yin[:, H:2 * H],
                                xwp[:].rearrange("p j w -> p w j"),
                                axis=mybir.AxisListType.X, op=ALU.add)

        # coord-attention chain (tiny, f32)
        y1p = pss.tile([8, 2 * H], f32, tag="small")
        nc.tensor.matmul(y1p[:], crf('caw1_t'), yin[:], start=True, stop=True)
        y2 = big.tile([8, 2 * H], f32, tag="y2")
        nc.scalar.activation(y2[:], y1p[:], AF.Prelu, bias=cc('cab')[0:8, :],
                             scale=cc('cas')[0:8, :], alpha=cc('caa')[0:8, :])
        awp = pss.tile([C, W], f32, tag="small")
        nc.tensor.matmul(awp[:], crf('caww_t', rows=8), y2[:, H:2 * H],
                         start=True, stop=True)
        aw = big.tile([C, W], bf16, tag="aw")
        nc.scalar.activation(aw[:], awp[:], AF.Sigmoid, bias=zcol, scale=1.0)
        ahp = pss.tile([C, H], f32, tag="small")
        nc.tensor.matmul(ahp[:], crf('cawh_t', rows=8), y2[:, 0:H],
                         start=True, stop=True)
        ah = big.tile([C, H], bf16, tag="ah")
        nc.scalar.activation(ah[:], ahp[:], AF.Sigmoid, bias=zcol, scale=1.0)

        # finals: out = pre * aw(bcast h) * ah(bcast w); blocks split DVE/GPS
        aw_b = aw[:].unsqueeze(1).broadcast_to((C, BH, W))
        for j in range(NBLK):
            r0, r1 = j * BH, (j + 1) * BH
            o_t = op.tile([C, BH * W], bf16, tag="o")
            o3 = o_t[:].rearrange("p (h w) -> p h w", w=W)
            ah_b = ah[:, r0:r1].unsqueeze(2).broadcast_to((C, BH, W))
            nc.vector.tensor_tensor(o3[:], ac3[:, r0:r1, :], aw_b, op=ALU.mult)
            e2 = nc.vector if j % 8 == 1 else nc.gpsimd
            e2.tensor_tensor(o3[:], o3[:], ah_b, op=ALU.mult)
            nc.sync.dma_start(o_p[:, r0 * W:r1 * W], o_t[:])

    nc.compile()
    return nc


def _get_graph(key, dve_h, dve_w, pe_h, pe_w, colf, colb, ckf, ckb):
    if key not in _GRAPH_CACHE:
        _GRAPH_CACHE[key] = _build(dve_h, dve_w, pe_h, pe_w, colf, colb,
                                   ckf, ckb)
    return _GRAPH_CACHE[key]


# ------------------------------------------------------------------ interface
def _run(inputs, trace=False):
    x = np.ascontiguousarray(np.asarray(inputs['x'], np.float32))
    assert x.shape == (B, C, H, W)
    consts, colf, constb, colb, dve_h, dve_w, pe_h, pe_w, key = _fold(inputs)
    nc = _get_graph(key, dve_h, dve_w, pe_h, pe_w, colf, colb,
                    consts.shape[1], constb.shape[1])
    xb = x.astype(ml_dtypes.bfloat16)
    in_maps = []
    for i in range(N_CORES):
        in_maps.append({'x': xb[i].reshape(C, HW).copy(),
                        'consts': consts, 'constb': constb})
    res = run_bass_kernel_spmd(nc, in_maps, list(range(N_CORES)), trace=trace)
    out = np.stack([res.results[i]['out'].astype(np.float32).reshape(C, H, W)
                    for i in range(N_CORES)], axis=0)
    return out, res


def kernel(**inputs):
    out, _ = _run(inputs, trace=False)
    return out



# revision 7
# speedup vs baseline: 1.7703x; 1.7703x over previous
"""Trainium2 Bass kernel for nn_Axial_PFCU_Continuous (dense_cnn).

Math (per sample, C=96, H=W=128), folded host-side:
  m+l   = cA0 (.) x + sum_d chA[d] (.) shiftH(x,d) + sum_d cwA[d] (.) shiftW(x,d)
  z     = Wf~ @ (m+l) + anchor;  anchor = cB0 (.) x + 4 edge taps + bias
  pre   = PReLU(z, a)
  coord attention: spatial means of pre -> tiny matmuls -> sigmoid gates
  out   = pre * ah(c,h) * aw(c,w)

Sharding: pure data-parallel, 1 of 8 batch samples per NeuronCore.

Per core (C=96 partitions, H*W free):
  PE  : one bf16 matmul W'@x per 8-row block (W' = wfuse~.diag(cA0)+diag(cB0))
        plus 8 fp8e4 DoubleRow PAIR matmuls per block covering all 12 shift
        terms of m+l and the 4 dwconv edge taps (2 terms per matmul at 0.5
        cycles/row), reading a zero-padded fp8 image so shifts are pure AP
        offsets with no edge masking. Tiny per-block coord-att matmuls.
  ACT : PReLU psum evict (bias folded), per-block coord-att PReLU/Sigmoid
  DVE : xw column-sum accumulation, xh fold+reduce, final x aw gate pass
  GPS : in-place x ah gate per block (mid-pipeline), part of x̃ loads
"""
import sys
import math

sys.path.insert(0, '/opt/trn_rl_repo')

import numpy as np
import ml_dtypes
from contextlib import ExitStack

import concourse.bass as bass
import concourse.bacc as bacc
from concourse import mybir, tile
from concourse.bass_utils import run_bass_kernel_spmd

f32 = mybir.dt.float32
bf16 = mybir.dt.bfloat16
fp16 = mybir.dt.float16
f8 = mybir.dt.float8e4
ALU = mybir.AluOpType
AF = mybir.ActivationFunctionType
PM = mybir.MatmulPerfMode

B, C, H, W = 8, 96, 128, 128
HW = H * W
EPS = 1e-5
N_CORES = 8
MIP = 8

NBLK = 16           # h-blocks per sample; one 8-row psum chunk per block
BH = H // NBLK

_GRAPH_CACHE = {}


# ----------------------------------------------------------------- host folds
def _taps(w_taps, r):
    """offset -> (C,) coefficient for the integer-shift decomposition."""
    r = max(float(r), 1.0)
    K = w_taps.shape[1]
    d2w = {}
    for i in range(K):
        s = (i - K // 2) * r
        f = math.floor(s)
        frac = s - f
        for d, wt in ((int(f), 1.0 - frac), (int(f) + 1, frac)):
            if wt != 0.0:
                if d not in d2w:
                    d2w[d] = np.zeros(C, np.float64)
                d2w[d] = d2w[d] + wt * np.asarray(w_taps[:, i], np.float64)
    return {d: w for d, w in d2w.items() if abs(d) < H}


def _merge(a, b):
    out = dict(a)
    for d, w in b.items():
        out[d] = out.get(d, np.zeros(C, np.float64)) + w
    return out


def _pairs(offsets):
    """Group offsets into pairs (symmetric +-d together when possible).
    Odd leftover is paired with a zero-coefficient duplicate marker None."""
    offs = sorted(offsets, key=lambda d: (abs(d), d))
    out = []
    used = set()
    for d in offs:
        if d in used:
            continue
        if -d in offs and -d not in used and d != -d:
            out.append((d, -d) if d < 0 else (-d, d))
            used.add(d); used.add(-d)
    rest = [d for d in offs if d not in used]
    for i in range(0, len(rest) - 1, 2):
        out.append((rest[i], rest[i + 1]))
    if len(rest) % 2:
        out.append((rest[-1], None))
    return out


class _Pack:
    def __init__(self, rows=C):
        self.cols = {}
        self.parts = []
        self.pos = 0
        self.rows = rows

    def put(self, name, arr):
        arr = np.asarray(arr, np.float64)
        if arr.ndim == 1:
            arr = arr[:, None]
        pad = np.zeros((self.rows, arr.shape[1]), np.float64)
        pad[:arr.shape[0], :] = arr
        self.cols[name] = (self.pos, arr.shape[1])
        self.parts.append(pad)
        self.pos += arr.shape[1]

    def done(self, dt, min_cols=0):
        if self.pos < min_cols:
            self.put('_pad', np.zeros((self.rows, min_cols - self.pos)))
        return np.concatenate(self.parts, axis=1).astype(dt)


def _fold(inp):
    g = lambda k: np.asarray(inp[k], np.float64)
    hA = _merge(_taps(g('wh_m'), float(np.asarray(inp['r_m']))),
                _taps(g('wh_l'), float(np.asarray(inp['r_l']))))
    wA = _merge(_taps(g('ww_m'), float(np.asarray(inp['r_m']))),
                _taps(g('ww_l'), float(np.asarray(inp['r_l']))))
    hA[0] = hA.get(0, np.zeros(C)) + 2.0    # identity terms of m+l
    wA.setdefault(0, np.zeros(C))

    h_offs = tuple(d for d in sorted(hA) if d != 0)
    w_offs = tuple(d for d in sorted(wA) if d != 0)
    h_pairs = _pairs(h_offs)
    w_pairs = _pairs(w_offs)
    halo_h = max([1] + [abs(d) for d in h_offs])
    halo_w = max([1] + [abs(d) for d in w_offs])
    PH, PW = H + 2 * halo_h, W + 2 * halo_w

    sf = g('bnf_g') / np.sqrt(g('bnf_v') + EPS)
    wfuse_t = (g('w_fuse') * sf[:, None]).T.copy()      # (Cin, Cout) lhsT
    bf = g('bnf_b') - g('bnf_m') * sf

    ds = g('dg_g') / np.sqrt(g('dg_v') + EPS)
    db = g('dg_b') - g('dg_m') * ds
    dg_wh, dg_ww = g('dg_wh'), g('dg_ww')
    ehm1, eh0, ehp1 = ds * dg_wh[:, 0], ds * (dg_wh[:, 1] + 1.0), ds * dg_wh[:, 2]
    ewm1, ew0, ewp1 = ds * dg_ww[:, 0], ds * dg_ww[:, 1], ds * dg_ww[:, 2]

    cA0 = hA[0] + wA[0]
    cB0 = eh0 + ew0
    bz = bf + db

    cs = g('ca_g') / np.sqrt(g('ca_v') + EPS)
    cb = g('ca_b') - g('ca_m') * cs

    # f32 consts (biases, PReLU params, CA chain scalars)
    pkf = _Pack()
    pkf.put('bz', bz)
    pkf.put('act_a', g('act_a'))
    pkf.put('zero', np.zeros(C))
    pkf.put('cas', cs); pkf.put('cab', cb); pkf.put('caa', g('ca_a'))
    consts = pkf.done(np.float32)

    # bf16 consts (main stationary + CA stationaries); padded for PE warmers
    Wp = wfuse_t * cA0[:, None] + np.diag(cB0)
    pkb = _Pack()
    pkb.put('Wp', Wp)
    pkb.put('caw1_t', (g('ca_w1') / float(W)).T)   # (C, 8); 1/W mean fold
    pkb.put('cawh_t', g('ca_wh').T)                # (8, C)
    pkb.put('caww_t', g('ca_ww').T)
    constb = pkb.done(ml_dtypes.bfloat16, min_cols=512)

    # fp8 pair stationaries: [lhsT(d1) | lhsT(d2)] per pair, 192 cols each
    def pairblk(cmap, pairs, diag):
        blocks = []
        for d1, d2 in pairs:
            for d in (d1, d2):
                coef = cmap[d] if d is not None else np.zeros(C)
                blocks.append(np.diag(coef) if diag else
                              wfuse_t * np.asarray(coef)[:, None])
        return blocks

    pk8 = _Pack()
    for i, blkpair in enumerate(_chunks(pairblk(hA, h_pairs, False), 2)):
        pk8.put(f'AH{i}', np.concatenate(blkpair, axis=1))
    for i, blkpair in enumerate(_chunks(pairblk(wA, w_pairs, False), 2)):
        pk8.put(f'AW{i}', np.concatenate(blkpair, axis=1))
    pk8.put('BH0', np.concatenate(
        [np.diag(ehm1), np.diag(ehp1)], axis=1))
    pk8.put('BW0', np.concatenate(
        [np.diag(ewm1), np.diag(ewp1)], axis=1))
    constf8 = pk8.done(ml_dtypes.float8_e4m3)

    bh_pairs = [(-1, 1)]
    bw_pairs = [(-1, 1)]
    key = (tuple(h_pairs), tuple(w_pairs), PH, PW,
           consts.shape[1], constb.shape[1], constf8.shape[1])
    meta = dict(h_pairs=h_pairs, w_pairs=w_pairs,
                bh_pairs=bh_pairs, bw_pairs=bw_pairs,
                PH=PH, PW=PW, halo_h=halo_h, halo_w=halo_w,
                colf=pkf.cols, colb=pkb.cols, col8=pk8.cols,
                ckf=consts.shape[1], ckb=constb.shape[1],
                ck8=constf8.shape[1])
    return consts, constb, constf8, meta, key


def _chunks(lst, n):
    return [lst[i:i + n] for i in range(0, len(lst), n)]


# -------------------------------------------------------------- graph builder
def _build(meta):
    h_pairs, w_pairs = meta['h_pairs'], meta['w_pairs']
    bh_pairs, bw_pairs = meta['bh_pairs'], meta['bw_pairs']
    PH, PW = meta['PH'], meta['PW']
    halo_h, halo_w = meta['halo_h'], meta['halo_w']
    colf, colb, col8 = meta['colf'], meta['colb'], meta['col8']

    nc = bacc.Bacc()
    x_p = nc.declare_dram_parameter("x", (C, HW), bf16, isOutput=False)
    x8_p = nc.declare_dram_parameter("x8", (C, PH * PW), f8, isOutput=False)
    cf_p = nc.declare_dram_parameter("consts", (C, meta['ckf']), f32,
                                     isOutput=False)
    cb_p = nc.declare_dram_parameter("constb", (C, meta['ckb']), bf16,
                                     isOutput=False)
    c8_p = nc.declare_dram_parameter("constf8", (C, meta['ck8']), f8,
                                     isOutput=False)
    o_p = nc.declare_dram_parameter("out", (C, HW), bf16, isOutput=True)

    with tile.TileContext(nc) as tc, ExitStack() as ctx:
        big = ctx.enter_context(tc.tile_pool(name="big", bufs=1))
        op = ctx.enter_context(tc.tile_pool(name="op", bufs=4))
        sm = ctx.enter_context(tc.tile_pool(name="sm", bufs=4))
        psq = ctx.enter_context(tc.tile_pool(name="psq", bufs=4, space="PSUM"))
        pss = ctx.enter_context(tc.tile_pool(name="pss", bufs=2, space="PSUM"))

        cst = big.tile([C, meta['ckf']], f32, tag="cst")
        cbt = big.tile([C, meta['ckb']], bf16, tag="cbt")
        c8t = big.tile([C, meta['ck8']], f8, tag="c8t")
        x_sb = big.tile([C, HW], bf16, tag="x")
        x8_sb = big.tile([C, PH * PW], f8, tag="x8")

        # --- DMA schedule -------------------------------------------------
        # x8 pieces: padded-row groups, contiguous in DRAM and SBUF.
        rpp = max(1, 1600 // PW)             # ~1.6KB pieces
        np8 = (PH + rpp - 1) // rpp
        p8 = []
        for p in range(np8):
            a = p * rpp * PW
            b = min((p + 1) * rpp, PH) * PW
            p8.append((a, b))
        # consts first, then interleave x chunks with early x8 pieces on SP;
        # late x8 pieces go to the gpsimd SWDGE queue (idle during fill).
        nc.sync.dma_start(cst[:], cf_p[:])
        nc.sync.dma_start(cbt[:], cb_p[:])
        nc.sync.dma_start(c8t[:], c8_p[:])
        n_sp8 = min(6, np8)
        sp_seq = []
        sp_seq.append(('x', 0))
        for p in range(n_sp8):
            sp_seq.append(('p8', p))
            if p % 2 == 1:
                sp_seq.append(('x', p // 2 + 1))
        done_x = 1 + n_sp8 // 2
        for j in range(done_x, NBLK):
            sp_seq.append(('x', j))
        for kind, idx in sp_seq:
            if kind == 'x':
                sl = slice(idx * HW // NBLK, (idx + 1) * HW // NBLK)
                nc.sync.dma_start(x_sb[:, sl], x_p[:, sl])
            else:
                a, b = p8[idx]
                nc.sync.dma_start(x8_sb[:, a:b], x8_p[:, a:b])
        for p in range(n_sp8, np8):
            a, b = p8[p]
            nc.gpsimd.dma_start(x8_sb[:, a:b], x8_p[:, a:b])

        def cc(name, rows=C):
            p0, n = colf[name]
            return cst[0:rows, p0:p0 + 1]

        def cb_(name, rows=C):
            p0, n = colb[name]
            return cbt[0:rows, p0:p0 + n]

        def c8_(name):
            p0, n = col8[name]
            return c8t[:, p0:p0 + n].rearrange("p (two m) -> p two m", two=2)

        zcol = cc('zero')

        # --- warmup: ACT tables once; PE p-state ramp during fill ---------
        wrm = sm.tile([C, 4], f32, tag="wrm")
        nc.scalar.activation(wrm[:, 0:1], zcol, AF.Prelu, bias=zcol,
                             scale=1.0, alpha=cc('act_a'))
        nc.scalar.activation(wrm[:, 1:2], zcol, AF.Sigmoid, bias=zcol,
                             scale=1.0)
        psw = pss.tile([C, 512], f32, tag="small")
        for wi in range(7):
            nc.tensor.matmul(psw[:], cbt[0:C, 0:C], cbt[0:C, 0:512],
                             start=(wi == 0), stop=(wi == 6))

        ac_sb = big.tile([C, HW], bf16, tag="ac")
        ac3 = ac_sb[:].rearrange("p (h w) -> p h w", w=W)
        x8b = x8_sb[:]

        xw4 = big.tile([C, 4 * W], fp16, tag="xw4")
        nc.vector.memset(xw4[:], 0)
        yin = big.tile([C, H], bf16, tag="yin")
        yinw = big.tile([C, W], bf16, tag="yinw")
        ah = big.tile([C, H], bf16, tag="ah")

        CH = 4              # psum chunk rows (512 f32 cols = one PSUM bank)

        def pair_rhs(r0, pair, axis):
            """Manual AP: (part, 2, CH rows, W cols) windows of padded x8."""
            d1, d2 = pair
            if d2 is None:
                d2 = d1
            if axis == 'h':
                o1 = (halo_h + r0 + d1) * PW + halo_w
                o2 = (halo_h + r0 + d2) * PW + halo_w
            else:
                o1 = (halo_h + r0) * PW + halo_w + d1
                o2 = (halo_h + r0) * PW + halo_w + d2
            ap_list = [list(x8b.ap[0]), [int(o2 - o1), 2], [PW, CH], [1, W]]
            return bass.AP(tensor=x8b.tensor, offset=x8b.offset + o1,
                           ap=ap_list)

        mms = ([(f'AH{i}', p, 'h') for i, p in enumerate(h_pairs)]
               + [(f'AW{i}', p, 'w') for i, p in enumerate(w_pairs)]
               + [('BH0', bh_pairs[0], 'h'), ('BW0', bw_pairs[0], 'w')])

        # --- main pipeline: one 8-row block (two 4-row chunks) per iter ---
        for j in range(NBLK):
            r0 = j * BH
            for k in range(BH // CH):
                cr0 = r0 + k * CH
                sl = slice(cr0 * W, (cr0 + CH) * W)
                pk = psq.tile([C, CH, W], f32, tag="pk")
                nc.tensor.matmul(pk[:], cb_('Wp'), x_sb[:, sl],
                                 start=True, stop=False)
                for mi, (nm, pair, axis) in enumerate(mms):
                    nc.tensor.matmul(pk[:], c8_(nm),
                                     pair_rhs(cr0, pair, axis),
                                     start=False, stop=(mi == len(mms) - 1),
                                     perf_mode=PM.DoubleRow)
                # evict with bias + PReLU
                nc.scalar.activation(ac_sb[:, sl], pk[:], AF.Prelu,
                                     bias=cc('bz'), scale=1.0,
                                     alpha=cc('act_a'))
                # xw column sums (fp16 accumulator, mod-4 rows)
                nc.vector.tensor_tensor(xw4[:], xw4[:], ac_sb[:, sl],
                                        op=ALU.add)
            # xh row sums: fold halves then reduce
            fold = sm.tile([C, BH * (W // 2)], bf16, tag="fold")
            f3 = fold[:].rearrange("p (h w) -> p h w", w=W // 2)
            nc.vector.tensor_tensor(f3[:], ac3[:, r0:r0 + BH, 0:W // 2],
                                    ac3[:, r0:r0 + BH, W // 2:W], op=ALU.add)
            with nc.allow_low_precision(reason="xh sums feed smooth gates"):
                nc.vector.tensor_reduce(yin[:, r0:r0 + BH], f3[:],
                                        axis=mybir.AxisListType.X, op=ALU.add)

            # incremental ah gate for this block (tiny matmuls + ACT)
            y1b = pss.tile([MIP, BH], f32, tag="small")
            nc.tensor.matmul(y1b[:], cb_('caw1_t'), yin[:, r0:r0 + BH],
                             start=True, stop=True)
            y2b = sm.tile([MIP, BH], bf16, tag="y2b")
            nc.scalar.activation(y2b[:], y1b[:], AF.Prelu,
                                 bias=cc('cab', rows=MIP),
                                 scale=cc('cas', rows=MIP),
                                 alpha=cc('caa', rows=MIP))
            ahp = pss.tile([C, BH], f32, tag="small")
            nc.tensor.matmul(ahp[:], cb_('cawh_t', rows=MIP), y2b[:],
                             start=True, stop=True)
            nc.scalar.activation(ah[:, r0:r0 + BH], ahp[:], AF.Sigmoid,
                                 bias=zcol, scale=1.0)
            # apply ah in place on the idle gpsimd engine
            ah_b = ah[:, r0:r0 + BH].unsqueeze(2).broadcast_to((C, BH, W))
            nc.gpsimd.tensor_tensor(ac3[:, r0:r0 + BH, :],
                                    ac3[:, r0:r0 + BH, :], ah_b, op=ALU.mult)

        # --- tail: aw gate ------------------------------------------------
        with nc.allow_low_precision(reason="xw sums feed smooth gates"):
            nc.vector.tensor_reduce(yinw[:],
                                    xw4[:].rearrange("p (j w) -> p w j", w=W),
                                    axis=mybir.AxisListType.X, op=ALU.add)
        y1w = pss.tile([MIP, W], f32, tag="small")
        nc.tensor.matmul(y1w[:], cb_('caw1_t'), yinw[:], start=True, stop=True)
        y2w = sm.tile([MIP, W], bf16, tag="y2w")
        nc.scalar.activation(y2w[:], y1w[:], AF.Prelu,
                             bias=cc('cab', rows=MIP),
                             scale=cc('cas', rows=MIP),
                             alpha=cc('caa', rows=MIP))
        awp = pss.tile([C, W], f32, tag="small")
        nc.tensor.matmul(awp[:], cb_('caww_t', rows=MIP), y2w[:],
                         start=True, stop=True)
        aw = sm.tile([C, W], bf16, tag="aw")
        nc.scalar.activation(aw[:], awp[:], AF.Sigmoid, bias=zcol, scale=1.0)

        aw_b = aw[:].unsqueeze(1).broadcast_to((C, BH, W))
        for j in range(NBLK):
            r0 = j * BH
            o_t = op.tile([C, BH * W], bf16, tag="o")
            o3 = o_t[:].rearrange("p (h w) -> p h w", w=W)
            eng = nc.gpsimd if j % 5 == 4 else nc.vector
            eng.tensor_tensor(o3[:], ac3[:, r0:r0 + BH, :], aw_b, op=ALU.mult)
            q = nc.sync if j % 2 == 0 else nc.scalar
            q.dma_start(o_p[:, r0 * W:(r0 + BH) * W], o_t[:])

    nc.compile()
    return nc


def _get_graph(meta, key):
    if key not in _GRAPH_CACHE:
        _GRAPH_CACHE[key] = _build(meta)
    return _GRAPH_CACHE[key]


# ------------------------------------------------------------------ interface
def _run(inputs, trace=False):
    x = np.ascontiguousarray(np.asarray(inputs['x'], np.float32))
    assert x.shape == (B, C, H, W)
    consts, constb, constf8, meta, key = _fold(inputs)
    nc = _get_graph(meta, key)
    PH, PW = meta['PH'], meta['PW']
    hh, hw = meta['halo_h'], meta['halo_w']
    xb = x.astype(ml_dtypes.bfloat16)
    xpad = np.zeros((B, C, PH, PW), ml_dtypes.float8_e4m3)
    xpad[:, :, hh:hh + H, hw:hw + W] = x.astype(ml_dtypes.float8_e4m3)
    in_maps = []
    for i in range(N_CORES):
        in_maps.append({'x': xb[i].reshape(C, HW).copy(),
                        'x8': xpad[i].reshape(C, PH * PW).copy(),
                        'consts': consts, 'constb': constb,
                        'constf8': constf8})
    res = run_bass_kernel_spmd(nc, in_maps, list(range(N_CORES)), trace=trace)
    out = np.stack([res.results[i]['out'].astype(np.float32).reshape(C, H, W)
                    for i in range(N_CORES)], axis=0)
    return out, res


def kernel(**inputs):
    out, _ = _run(inputs, trace=False)
    return out


# revision 16
# speedup vs baseline: 2.0436x; 1.1543x over previous
"""Trainium2 Bass kernel for nn_Axial_PFCU_Continuous (dense_cnn).

Math (per sample, C=96, H=W=128), folded host-side:
  m+l   = cA0 (.) x + sum_d chA[d] (.) shiftH(x,d) + sum_d cwA[d] (.) shiftW(x,d)
  z     = Wf~ @ (m+l) + anchor;  anchor = cB0 (.) x + 4 edge taps + bias
  pre   = PReLU(z, a)
  coord attention: spatial means of pre -> tiny matmuls -> sigmoid gates
  out   = pre * ah(c,h) * aw(c,w)

Sharding: pure data-parallel, 1 of 8 batch samples per NeuronCore.

Per core (C=96 partitions, H*W free):
  PE  : one bf16 matmul W'@x per 8-row block (W' = wfuse~.diag(cA0)+diag(cB0))
        plus 8 fp8e4 DoubleRow PAIR matmuls per block covering all 12 shift
        terms of m+l and the 4 dwconv edge taps (2 terms per matmul at 0.5
        cycles/row), reading a zero-padded fp8 image so shifts are pure AP
        offsets with no edge masking. Tiny per-block coord-att matmuls.
  ACT : PReLU psum evict (bias folded), per-block coord-att PReLU/Sigmoid
  DVE : xw column-sum accumulation, xh fold+reduce, final x aw gate pass
  GPS : in-place x ah gate per block (mid-pipeline), part of x̃ loads
"""
import sys
import math

sys.path.insert(0, '/opt/trn_rl_repo')

import numpy as np
import ml_dtypes
from contextlib import ExitStack

import concourse.bass as bass
import concourse.bacc as bacc
from concourse import mybir, tile
from concourse.bass_utils import run_bass_kernel_spmd

f32 = mybir.dt.float32
bf16 = mybir.dt.bfloat16
fp16 = mybir.dt.float16
f8 = mybir.dt.float8e4
ALU = mybir.AluOpType
AF = mybir.ActivationFunctionType
PM = mybir.MatmulPerfMode

B, C, H, W = 8, 96, 128, 128
HW = H * W
EPS = 1e-5
N_CORES = 8
MIP = 8

NBLK = 16           # h-blocks per sample; one 8-row psum chunk per block
BH = H // NBLK

_GRAPH_CACHE = {}


# ----------------------------------------------------------------- host folds
def _taps(w_taps, r):
    """offset -> (C,) coefficient for the integer-shift decomposition."""
    r = max(float(r), 1.0)
    K = w_taps.shape[1]
    d2w = {}
    for i in range(K):
        s = (i - K // 2) * r
        f = math.floor(s)
        frac = s - f
        for d, wt in ((int(f), 1.0 - frac), (int(f) + 1, frac)):
            if wt != 0.0:
                if d not in d2w:
                    d2w[d] = np.zeros(C, np.float64)
                d2w[d] = d2w[d] + wt * np.asarray(w_taps[:, i], np.float64)
    return {d: w for d, w in d2w.items() if abs(d) < H}


def _merge(a, b):
    out = dict(a)
    for d, w in b.items():
        out[d] = out.get(d, np.zeros(C, np.float64)) + w
    return out


def _pairs(offsets):
    """Group offsets into pairs (symmetric +-d together when possible).
    Odd leftover is paired with a zero-coefficient duplicate marker None."""
    offs = sorted(offsets, key=lambda d: (abs(d), d))
    out = []
    used = set()
    for d in offs:
        if d in used:
            continue
        if -d in offs and -d not in used and d != -d:
            out.append((d, -d) if d < 0 else (-d, d))
            used.add(d); used.add(-d)
    rest = [d for d in offs if d not in used]
    for i in range(0, len(rest) - 1, 2):
        out.append((rest[i], rest[i + 1]))
    if len(rest) % 2:
        out.append((rest[-1], None))
    return out


class _Pack:
    def __init__(self, rows=C):
        self.cols = {}
        self.parts = []
        self.pos = 0
        self.rows = rows

    def put(self, name, arr):
        arr = np.asarray(arr, np.float64)
        if arr.ndim == 1:
            arr = arr[:, None]
        pad = np.zeros((self.rows, arr.shape[1]), np.float64)
        pad[:arr.shape[0], :] = arr
        self.cols[name] = (self.pos, arr.shape[1])
        self.parts.append(pad)
        self.pos += arr.shape[1]

    def done(self, dt, min_cols=0):
        if self.pos < min_cols:
            self.put('_pad', np.zeros((self.rows, min_cols - self.pos)))
        return np.concatenate(self.parts, axis=1).astype(dt)


def _fold(inp):
    g = lambda k: np.asarray(inp[k], np.float64)
    hA = _merge(_taps(g('wh_m'), float(np.asarray(inp['r_m']))),
                _taps(g('wh_l'), float(np.asarray(inp['r_l']))))
    wA = _merge(_taps(g('ww_m'), float(np.asarray(inp['r_m']))),
                _taps(g('ww_l'), float(np.asarray(inp['r_l']))))
    hA[0] = hA.get(0, np.zeros(C)) + 2.0    # identity terms of m+l
    wA.setdefault(0, np.zeros(C))

    h_offs = tuple(d for d in sorted(hA) if d != 0)
    w_offs = tuple(d for d in sorted(wA) if d != 0)
    h_pairs = _pairs(h_offs)
    w_pairs = _pairs(w_offs)
    halo_h = max([1] + [abs(d) for d in h_offs])
    halo_w = max([1] + [abs(d) for d in w_offs])
    PH, PW = H + 2 * halo_h, W + 2 * halo_w

    sf = g('bnf_g') / np.sqrt(g('bnf_v') + EPS)
    wfuse_t = (g('w_fuse') * sf[:, None]).T.copy()      # (Cin, Cout) lhsT
    bf = g('bnf_b') - g('bnf_m') * sf

    ds = g('dg_g') / np.sqrt(g('dg_v') + EPS)
    db = g('dg_b') - g('dg_m') * ds
    dg_wh, dg_ww = g('dg_wh'), g('dg_ww')
    ehm1, eh0, ehp1 = ds * dg_wh[:, 0], ds * (dg_wh[:, 1] + 1.0), ds * dg_wh[:, 2]
    ewm1, ew0, ewp1 = ds * dg_ww[:, 0], ds * dg_ww[:, 1], ds * dg_ww[:, 2]

    cA0 = hA[0] + wA[0]
    cB0 = eh0 + ew0
    bz = bf + db

    cs = g('ca_g') / np.sqrt(g('ca_v') + EPS)
    cb = g('ca_b') - g('ca_m') * cs

    # f32 consts (biases, PReLU params, CA chain scalars)
    pkf = _Pack()
    pkf.put('bz', bz)
    pkf.put('act_a', g('act_a'))
    pkf.put('zero', np.zeros(C))
    pkf.put('cas', cs); pkf.put('cab', cb); pkf.put('caa', g('ca_a'))
    consts = pkf.done(np.float32)

    # bf16 consts (main stationary + CA stationaries); padded for PE warmers
    Wp = wfuse_t * cA0[:, None] + np.diag(cB0)
    pkb = _Pack()
    pkb.put('Wp', Wp)
    pkb.put('caw1_t', (g('ca_w1') / float(W)).T)   # (C, 8); 1/W mean fold
    pkb.put('cawh_t', g('ca_wh').T)                # (8, C)
    pkb.put('caww_t', g('ca_ww').T)
    constb = pkb.done(ml_dtypes.bfloat16, min_cols=512)

    # per-channel fp8 scale s[c] = 8*u[c]: the 8x lifts the folded wfuse*chA
    # stationaries out of subnormal range; u is grid-tuned so the four edge
    # taps quantize well (the A rows are insensitive to the exact scale).
    # x8 carries x/s; stationary rows carry *s, so products are exact in s.
    f8r = lambda a: np.asarray(a, np.float32).astype(
        ml_dtypes.float8_e4m3).astype(np.float64)
    taps4 = np.stack([ehm1, ehp1, ewm1, ewp1], 0)
    u = np.ones(C)
    cands = np.linspace(0.6, 1.45, 160)
    for c in range(C):
        errs = [np.sum((f8r(taps4[:, c] * 8 * uu) / (8 * uu)
                        - taps4[:, c]) ** 2) for uu in cands]
        u[c] = cands[int(np.argmin(errs))]
    s8 = 8.0 * u

    # fp8 pair stationaries: [lhsT(d1) | lhsT(d2)] per pair, 192 cols each
    def pairblk(cmap, pairs, diag):
        blocks = []
        for d1, d2 in pairs:
            for d in (d1, d2):
                coef = cmap[d] if d is not None else np.zeros(C)
                blocks.append(np.diag(coef * s8) if diag else
                              wfuse_t * (np.asarray(coef) * s8)[:, None])
        return blocks

    pk8 = _Pack()
    for i, blkpair in enumerate(_chunks(pairblk(hA, h_pairs, False), 2)):
        pk8.put(f'AH{i}', np.concatenate(blkpair, axis=1))
    for i, blkpair in enumerate(_chunks(pairblk(wA, w_pairs, False), 2)):
        pk8.put(f'AW{i}', np.concatenate(blkpair, axis=1))
    pk8.put('BH0', np.concatenate(
        [np.diag(ehm1 * s8), np.diag(ehp1 * s8)], axis=1))
    pk8.put('BW0', np.concatenate(
        [np.diag(ewm1 * s8), np.diag(ewp1 * s8)], axis=1))
    constf8 = pk8.done(ml_dtypes.float8_e4m3)

    bh_pairs = [(-1, 1)]
    bw_pairs = [(-1, 1)]
    key = (tuple(h_pairs), tuple(w_pairs), PH, PW,
           consts.shape[1], constb.shape[1], constf8.shape[1])
    meta = dict(h_pairs=h_pairs, w_pairs=w_pairs,
                bh_pairs=bh_pairs, bw_pairs=bw_pairs,
                PH=PH, PW=PW, halo_h=halo_h, halo_w=halo_w,
                colf=pkf.cols, colb=pkb.cols, col8=pk8.cols,
                ckf=consts.shape[1], ckb=constb.shape[1],
                ck8=constf8.shape[1], s8=s8)
    return consts, constb, constf8, meta, key


def _chunks(lst, n):
    return [lst[i:i + n] for i in range(0, len(lst), n)]


# -------------------------------------------------------------- graph builder
def _build(meta):
    h_pairs, w_pairs = meta['h_pairs'], meta['w_pairs']
    bh_pairs, bw_pairs = meta['bh_pairs'], meta['bw_pairs']
    PH, PW = meta['PH'], meta['PW']
    halo_h, halo_w = meta['halo_h'], meta['halo_w']
    colf, colb, col8 = meta['colf'], meta['colb'], meta['col8']

    nc = bacc.Bacc()
    x_p = nc.declare_dram_parameter("x", (C, HW), bf16, isOutput=False)
    x8_p = nc.declare_dram_parameter("x8", (C, PH * PW), f8, isOutput=False)
    cf_p = nc.declare_dram_parameter("consts", (C, meta['ckf']), f32,
                                     isOutput=False)
    cb_p = nc.declare_dram_parameter("constb", (C, meta['ckb']), bf16,
                                     isOutput=False)
    c8_p = nc.declare_dram_parameter("constf8", (C, meta['ck8']), f8,
                                     isOutput=False)
    o_p = nc.declare_dram_parameter("out", (C, HW), bf16, isOutput=True)

    with tile.TileContext(nc) as tc, ExitStack() as ctx:
        big = ctx.enter_context(tc.tile_pool(name="big", bufs=1))
        sm = ctx.enter_context(tc.tile_pool(name="sm", bufs=4))
        psq = ctx.enter_context(tc.tile_pool(name="psq", bufs=3, space="PSUM"))
        pss = ctx.enter_context(tc.tile_pool(name="pss", bufs=2, space="PSUM"))

        cst = big.tile([C, meta['ckf']], f32, tag="cst")
        cbt = big.tile([C, meta['ckb']], bf16, tag="cbt")
        c8t = big.tile([C, meta['ck8']], f8, tag="c8t")
        x_sb = big.tile([C, HW], bf16, tag="x")
        x8_sb = big.tile([C, PH * PW], f8, tag="x8")

        # --- DMA schedule -------------------------------------------------
        # x8 pieces: padded-row groups, contiguous in DRAM and SBUF.
        rpp = max(1, 1600 // PW)             # ~1.6KB pieces
        np8 = (PH + rpp - 1) // rpp
        p8 = []
        for p in range(np8):
            a = p * rpp * PW
            b = min((p + 1) * rpp, PH) * PW
            p8.append((a, b))
        # consts on the ACT queue (idle early; warmers only need cbt which
        # lands first); x chunks + early x8 pieces interleave on SP; the
        # first halo pieces and the late pieces go via the gpsimd SWDGE.
        nc.scalar.dma_start(cbt[:], cb_p[:])
        nc.scalar.dma_start(cst[:], cf_p[:])
        nc.scalar.dma_start(c8t[:], c8_p[:])
        sp_seq = [('x', 0), ('p8', 0), ('p8', 1), ('x', 1), ('p8', 4),
                  ('x', 2), ('p8', 5), ('x', 3), ('p8', 6), ('x', 4),
                  ('p8', 7), ('x', 5)] + [('x', j) for j in range(6, NBLK)]
        for kind, idx in sp_seq:
            if kind == 'x':
                sl = slice(idx * HW // NBLK, (idx + 1) * HW // NBLK)
                nc.sync.dma_start(x_sb[:, sl], x_p[:, sl])
            elif idx < np8:
                a, b = p8[idx]
                nc.sync.dma_start(x8_sb[:, a:b], x8_p[:, a:b])
        for p in list(range(2, 4)) + list(range(8, np8)):
            a, b = p8[p]
            nc.gpsimd.dma_start(x8_sb[:, a:b], x8_p[:, a:b])

        def cc(name, rows=C):
            p0, n = colf[name]
            return cst[0:rows, p0:p0 + 1]

        def cb_(name, rows=C):
            p0, n = colb[name]
            return cbt[0:rows, p0:p0 + n]

        def c8_(name):
            p0, n = col8[name]
            return c8t[:, p0:p0 + n].rearrange("p (two m) -> p two m", two=2)

        zcol = cc('zero')

        # --- warmup: ACT tables once; PE p-state ramp during fill ---------
        wrm = sm.tile([C, 4], f32, tag="wrm")
        nc.scalar.activation(wrm[:, 0:1], zcol, AF.Prelu, bias=zcol,
                             scale=1.0, alpha=cc('act_a'))
        nc.scalar.activation(wrm[:, 1:2], zcol, AF.Sigmoid, bias=zcol,
                             scale=1.0)
        psw = pss.tile([C, 512], f32, tag="small")
        for wi in range(4):
            nc.tensor.matmul(psw[:], cbt[0:C, 0:C], cbt[0:C, 0:512],
                             start=(wi == 0), stop=(wi == 3))

        ac_sb = big.tile([C, HW], bf16, tag="ac")
        ac3 = ac_sb[:].rearrange("p (h w) -> p h w", w=W)
        x8b = x8_sb[:]

        xw4 = big.tile([C, 4 * W], fp16, tag="xw4")
        nc.vector.memset(xw4[:], 0)
        yin = big.tile([C, H], bf16, tag="yin")
        yinw = big.tile([C, W], bf16, tag="yinw")
        ah = big.tile([C, H], bf16, tag="ah")

        CH = 4              # psum chunk rows (512 f32 cols = one PSUM bank)

        def pair_rhs(r0, pair, axis):
            """Manual AP: (part, 2, CH rows, W cols) windows of padded x8."""
            d1, d2 = pair
            if d2 is None:
                d2 = d1
            if axis == 'h':
                o1 = (halo_h + r0 + d1) * PW + halo_w
                o2 = (halo_h + r0 + d2) * PW + halo_w
            else:
                o1 = (halo_h + r0) * PW + halo_w + d1
                o2 = (halo_h + r0) * PW + halo_w + d2
            ap_list = [list(x8b.ap[0]), [int(o2 - o1), 2], [PW, CH], [1, W]]
            return bass.AP(tensor=x8b.tensor, offset=x8b.offset + o1,
                           ap=ap_list)

        mms = ([(f'AH{i}', p, 'h') for i, p in enumerate(h_pairs)]
               + [(f'AW{i}', p, 'w') for i, p in enumerate(w_pairs)]
               + [('BH0', bh_pairs[0], 'h'), ('BW0', bw_pairs[0], 'w')])

        # --- main pipeline: one 8-row block (two 4-row psum groups sharing
        # a 2-bank psum tile, single merged evict) per iteration -----------
        for j in range(NBLK):
            r0 = j * BH
            sl = slice(r0 * W, (r0 + BH) * W)
            pk = psq.tile([C, BH, W], f32, tag="pk")
            for k in range(BH // CH):
                cr0 = r0 + k * CH
                pkk = pk[:, k * CH:(k + 1) * CH, :]
                nc.tensor.matmul(pkk, cb_('Wp'),
                                 x_sb[:, cr0 * W:(cr0 + CH) * W],
                                 start=True, stop=False)
                for mi, (nm, pair, axis) in enumerate(mms):
                    nc.tensor.matmul(pkk, c8_(nm),
                                     pair_rhs(cr0, pair, axis),
                                     start=False, stop=(mi == len(mms) - 1),
                                     perf_mode=PM.DoubleRow)
            # evict with bias + PReLU (whole block, one ACT op)
            nc.scalar.activation(ac_sb[:, sl], pk[:], AF.Prelu,
                                 bias=cc('bz'), scale=1.0, alpha=cc('act_a'))
            # xw column sums (fp16 accumulator, mod-4 rows)
            half = CH * W
            nc.vector.tensor_tensor(xw4[:], xw4[:],
                                    ac_sb[:, r0 * W:r0 * W + half],
                                    op=ALU.add)
            nc.vector.tensor_tensor(xw4[:], xw4[:],
                                    ac_sb[:, r0 * W + half:(r0 + BH) * W],
                                    op=ALU.add)
            # xh row sums: fold halves then reduce
            fold = sm.tile([C, BH * (W // 2)], bf16, tag="fold")
            f3 = fold[:].rearrange("p (h w) -> p h w", w=W // 2)
            nc.vector.tensor_tensor(f3[:], ac3[:, r0:r0 + BH, 0:W // 2],
                                    ac3[:, r0:r0 + BH, W // 2:W], op=ALU.add)
            with nc.allow_low_precision(reason="xh sums feed smooth gates"):
                nc.vector.tensor_reduce(yin[:, r0:r0 + BH], f3[:],
                                        axis=mybir.AxisListType.X, op=ALU.add)

            # incremental ah gate for this block (tiny matmuls + ACT)
            y1b = pss.tile([MIP, BH], f32, tag="small")
            nc.tensor.matmul(y1b[:], cb_('caw1_t'), yin[:, r0:r0 + BH],
                             start=True, stop=True)
            y2b = sm.tile([MIP, BH], bf16, tag="y2b")
            nc.scalar.activation(y2b[:], y1b[:], AF.Prelu,
                                 bias=cc('cab', rows=MIP),
                                 scale=cc('cas', rows=MIP),
                                 alpha=cc('caa', rows=MIP))
            ahp = pss.tile([C, BH], f32, tag="small")
            nc.tensor.matmul(ahp[:], cb_('cawh_t', rows=MIP), y2b[:],
                             start=True, stop=True)
            nc.scalar.activation(ah[:, r0:r0 + BH], ahp[:], AF.Sigmoid,
                                 bias=zcol, scale=1.0)
            # apply ah in place on the idle gpsimd engine
            ah_b = ah[:, r0:r0 + BH].unsqueeze(2).broadcast_to((C, BH, W))
            nc.gpsimd.tensor_tensor(ac3[:, r0:r0 + BH, :],
                                    ac3[:, r0:r0 + BH, :], ah_b, op=ALU.mult)

        # --- tail: aw gate (xw reduced via two folds, 2x DVE mode) --------
        xwA = sm.tile([C, 2 * W], fp16, tag="xwA")
        nc.vector.tensor_tensor(xwA[:], xw4[:, 0:2 * W], xw4[:, 2 * W:4 * W],
                                op=ALU.add)
        nc.vector.tensor_tensor(yinw[:], xwA[:, 0:W], xwA[:, W:2 * W],
                                op=ALU.add)
        y1w = pss.tile([MIP, W], f32, tag="small")
        nc.tensor.matmul(y1w[:], cb_('caw1_t'), yinw[:], start=True, stop=True)
        y2w = sm.tile([MIP, W], bf16, tag="y2w")
        nc.scalar.activation(y2w[:], y1w[:], AF.Prelu,
                             bias=cc('cab', rows=MIP),
                             scale=cc('cas', rows=MIP),
                             alpha=cc('caa', rows=MIP))
        awp = pss.tile([C, W], f32, tag="small")
        nc.tensor.matmul(awp[:], cb_('caww_t', rows=MIP), y2w[:],
                         start=True, stop=True)
        aw = sm.tile([C, W], bf16, tag="aw")
        nc.scalar.activation(aw[:], awp[:], AF.Sigmoid, bias=zcol, scale=1.0)

        aw_b = aw[:].unsqueeze(1).broadcast_to((C, BH, W))
        for j in range(NBLK):
            r0 = j * BH
            eng = nc.gpsimd if (j % 3 == 1 or j == 15) else nc.vector
            eng.tensor_tensor(ac3[:, r0:r0 + BH, :], ac3[:, r0:r0 + BH, :],
                              aw_b, op=ALU.mult)
            q = nc.sync if j % 2 == 0 else nc.scalar
            q.dma_start(o_p[:, r0 * W:(r0 + BH) * W],
                        ac_sb[:, r0 * W:(r0 + BH) * W])

    nc.compile()
    return nc


def _get_graph(meta, key):
    if key not in _GRAPH_CACHE:
        _GRAPH_CACHE[key] = _build(meta)
    return _GRAPH_CACHE[key]


# ------------------------------------------------------------------ interface
def _run(inputs, trace=False):
    x = np.ascontiguousarray(np.asarray(inputs['x'], np.float32))
    assert x.shape == (B, C, H, W)
    consts, constb, constf8, meta, key = _fold(inputs)
    nc = _get_graph(meta, key)
    PH, PW = meta['PH'], meta['PW']
    hh, hw = meta['halo_h'], meta['halo_w']
    xb = x.astype(ml_dtypes.bfloat16)
    xs = (x / meta['s8'][None, :, None, None].astype(np.float32))
    xpad = np.zeros((B, C, PH, PW), ml_dtypes.float8_e4m3)
    xpad[:, :, hh:hh + H, hw:hw + W] = xs.astype(ml_dtypes.float8_e4m3)
    in_maps = []
    for i in range(N_CORES):
        in_maps.append({'x': xb[i].reshape(C, HW).copy(),
                        'x8': xpad[i].reshape(C, PH * PW).copy(),
                        'consts': consts, 'constb': constb,
                        'constf8': constf8})
    res = run_bass_kernel_spmd(nc, in_maps, list(range(N_CORES)), trace=trace)
    out = np.stack([res.results[i]['out'].astype(np.float32).reshape(C, H, W)
                    for i in range(N_CORES)], axis=0)
    return out, res


def kernel(**inputs):
    out, _ = _run(inputs, trace=False)
    return out


# revision 19
# speedup vs baseline: 2.0698x; 1.0129x over previous
"""Trainium2 Bass kernel for nn_Axial_PFCU_Continuous (dense_cnn).

Math (per sample, C=96, H=W=128), folded host-side:
  m+l   = cA0 (.) x + sum_d chA[d] (.) shiftH(x,d) + sum_d cwA[d] (.) shiftW(x,d)
  z     = Wf~ @ (m+l) + anchor;  anchor = cB0 (.) x + 4 edge taps + bias
  pre   = PReLU(z, a)
  coord attention: spatial means of pre -> tiny matmuls -> sigmoid gates
  out   = pre * ah(c,h) * aw(c,w)

Sharding: pure data-parallel, 1 of 8 batch samples per NeuronCore.

Per core (C=96 partitions, H*W free):
  PE  : one bf16 matmul W'@x per 8-row block (W' = wfuse~.diag(cA0)+diag(cB0))
        plus 8 fp8e4 DoubleRow PAIR matmuls per block covering all 12 shift
        terms of m+l and the 4 dwconv edge taps (2 terms per matmul at 0.5
        cycles/row), reading a zero-padded fp8 image so shifts are pure AP
        offsets with no edge masking. Tiny per-block coord-att matmuls.
  ACT : PReLU psum evict (bias folded), per-block coord-att PReLU/Sigmoid
  DVE : xw column-sum accumulation, xh fold+reduce, final x aw gate pass
  GPS : in-place x ah gate per block (mid-pipeline), part of x̃ loads
"""
import sys
import math

sys.path.insert(0, '/opt/trn_rl_repo')

import numpy as np
import ml_dtypes
from contextlib import ExitStack

import concourse.bass as bass
import concourse.bacc as bacc
from concourse import mybir, tile
from concourse.bass_utils import run_bass_kernel_spmd

f32 = mybir.dt.float32
bf16 = mybir.dt.bfloat16
fp16 = mybir.dt.float16
f8 = mybir.dt.float8e4
ALU = mybir.AluOpType
AF = mybir.ActivationFunctionType
PM = mybir.MatmulPerfMode

B, C, H, W = 8, 96, 128, 128
HW = H * W
EPS = 1e-5
N_CORES = 8
MIP = 8

NBLK = 16           # h-blocks per sample; one 8-row psum chunk per block
BH = H // NBLK

_GRAPH_CACHE = {}


# ----------------------------------------------------------------- host folds
def _taps(w_taps, r):
    """offset -> (C,) coefficient for the integer-shift decomposition."""
    r = max(float(r), 1.0)
    K = w_taps.shape[1]
    d2w = {}
    for i in range(K):
        s = (i - K // 2) * r
        f = math.floor(s)
        frac = s - f
        for d, wt in ((int(f), 1.0 - frac), (int(f) + 1, frac)):
            if wt != 0.0:
                if d not in d2w:
                    d2w[d] = np.zeros(C, np.float64)
                d2w[d] = d2w[d] + wt * np.asarray(w_taps[:, i], np.float64)
    return {d: w for d, w in d2w.items() if abs(d) < H}


def _merge(a, b):
    out = dict(a)
    for d, w in b.items():
        out[d] = out.get(d, np.zeros(C, np.float64)) + w
    return out


def _pairs(offsets):
    """Group offsets into pairs (symmetric +-d together when possible).
    Odd leftover is paired with a zero-coefficient duplicate marker None."""
    offs = sorted(offsets, key=lambda d: (abs(d), d))
    out = []
    used = set()
    for d in offs:
        if d in used:
            continue
        if -d in offs and -d not in used and d != -d:
            out.append((d, -d) if d < 0 else (-d, d))
            used.add(d); used.add(-d)
    rest = [d for d in offs if d not in used]
    for i in range(0, len(rest) - 1, 2):
        out.append((rest[i], rest[i + 1]))
    if len(rest) % 2:
        out.append((rest[-1], None))
    return out


class _Pack:
    def __init__(self, rows=C):
        self.cols = {}
        self.parts = []
        self.pos = 0
        self.rows = rows

    def put(self, name, arr):
        arr = np.asarray(arr, np.float64)
        if arr.ndim == 1:
            arr = arr[:, None]
        pad = np.zeros((self.rows, arr.shape[1]), np.float64)
        pad[:arr.shape[0], :] = arr
        self.cols[name] = (self.pos, arr.shape[1])
        self.parts.append(pad)
        self.pos += arr.shape[1]

    def done(self, dt, min_cols=0):
        if self.pos < min_cols:
            self.put('_pad', np.zeros((self.rows, min_cols - self.pos)))
        return np.concatenate(self.parts, axis=1).astype(dt)


def _fold(inp):
    g = lambda k: np.asarray(inp[k], np.float64)
    hA = _merge(_taps(g('wh_m'), float(np.asarray(inp['r_m']))),
                _taps(g('wh_l'), float(np.asarray(inp['r_l']))))
    wA = _merge(_taps(g('ww_m'), float(np.asarray(inp['r_m']))),
                _taps(g('ww_l'), float(np.asarray(inp['r_l']))))
    hA[0] = hA.get(0, np.zeros(C)) + 2.0    # identity terms of m+l
    wA.setdefault(0, np.zeros(C))

    h_offs = tuple(d for d in sorted(hA) if d != 0)
    w_offs = tuple(d for d in sorted(wA) if d != 0)
    h_pairs = _pairs(h_offs)
    w_pairs = _pairs(w_offs)
    halo_h = max([1] + [abs(d) for d in h_offs])
    halo_w = max([1] + [abs(d) for d in w_offs])
    PH, PW = H + 2 * halo_h, W + 2 * halo_w

    sf = g('bnf_g') / np.sqrt(g('bnf_v') + EPS)
    wfuse_t = (g('w_fuse') * sf[:, None]).T.copy()      # (Cin, Cout) lhsT
    bf = g('bnf_b') - g('bnf_m') * sf

    ds = g('dg_g') / np.sqrt(g('dg_v') + EPS)
    db = g('dg_b') - g('dg_m') * ds
    dg_wh, dg_ww = g('dg_wh'), g('dg_ww')
    ehm1, eh0, ehp1 = ds * dg_wh[:, 0], ds * (dg_wh[:, 1] + 1.0), ds * dg_wh[:, 2]
    ewm1, ew0, ewp1 = ds * dg_ww[:, 0], ds * dg_ww[:, 1], ds * dg_ww[:, 2]

    cA0 = hA[0] + wA[0]
    cB0 = eh0 + ew0
    bz = bf + db

    cs = g('ca_g') / np.sqrt(g('ca_v') + EPS)
    cb = g('ca_b') - g('ca_m') * cs

    # f32 consts (biases, PReLU params, CA chain scalars)
    pkf = _Pack()
    pkf.put('bz', bz)
    pkf.put('act_a', g('act_a'))
    pkf.put('zero', np.zeros(C))
    pkf.put('cas', cs); pkf.put('cab', cb); pkf.put('caa', g('ca_a'))
    consts = pkf.done(np.float32)

    # bf16 consts (main stationary + CA stationaries); padded for PE warmers
    Wp = wfuse_t * cA0[:, None] + np.diag(cB0)
    pkb = _Pack()
    pkb.put('Wp', Wp)
    pkb.put('caw1_t', (g('ca_w1') / float(W)).T)   # (C, 8); 1/W mean fold
    pkb.put('cawh_t', g('ca_wh').T)                # (8, C)
    pkb.put('caww_t', g('ca_ww').T)
    constb = pkb.done(ml_dtypes.bfloat16, min_cols=512)

    # per-channel fp8 scale s[c] = 8*u[c]: the 8x lifts the folded wfuse*chA
    # stationaries out of subnormal range; u is grid-tuned so the four edge
    # taps quantize well (the A rows are insensitive to the exact scale).
    # x8 carries x/s; stationary rows carry *s, so products are exact in s.
    f8r = lambda a: np.asarray(a, np.float32).astype(
        ml_dtypes.float8_e4m3).astype(np.float64)
    taps4 = np.stack([ehm1, ehp1, ewm1, ewp1], 0)
    u = np.ones(C)
    cands = np.linspace(0.6, 1.45, 160)
    for c in range(C):
        errs = [np.sum((f8r(taps4[:, c] * 8 * uu) / (8 * uu)
                        - taps4[:, c]) ** 2) for uu in cands]
        u[c] = cands[int(np.argmin(errs))]
    s8 = 8.0 * u

    # fp8 pair stationaries: [lhsT(d1) | lhsT(d2)] per pair, 192 cols each
    def pairblk(cmap, pairs, diag):
        blocks = []
        for d1, d2 in pairs:
            for d in (d1, d2):
                coef = cmap[d] if d is not None else np.zeros(C)
                blocks.append(np.diag(coef * s8) if diag else
                              wfuse_t * (np.asarray(coef) * s8)[:, None])
        return blocks

    pk8 = _Pack()
    for i, blkpair in enumerate(_chunks(pairblk(hA, h_pairs, False), 2)):
        pk8.put(f'AH{i}', np.concatenate(blkpair, axis=1))
    for i, blkpair in enumerate(_chunks(pairblk(wA, w_pairs, False), 2)):
        pk8.put(f'AW{i}', np.concatenate(blkpair, axis=1))
    pk8.put('BH0', np.concatenate(
        [np.diag(ehm1 * s8), np.diag(ehp1 * s8)], axis=1))
    pk8.put('BW0', np.concatenate(
        [np.diag(ewm1 * s8), np.diag(ewp1 * s8)], axis=1))
    constf8 = pk8.done(ml_dtypes.float8_e4m3)

    bh_pairs = [(-1, 1)]
    bw_pairs = [(-1, 1)]
    key = (tuple(h_pairs), tuple(w_pairs), PH, PW,
           consts.shape[1], constb.shape[1], constf8.shape[1])
    meta = dict(h_pairs=h_pairs, w_pairs=w_pairs,
                bh_pairs=bh_pairs, bw_pairs=bw_pairs,
                PH=PH, PW=PW, halo_h=halo_h, halo_w=halo_w,
                colf=pkf.cols, colb=pkb.cols, col8=pk8.cols,
                ckf=consts.shape[1], ckb=constb.shape[1],
                ck8=constf8.shape[1], s8=s8)
    return consts, constb, constf8, meta, key


def _chunks(lst, n):
    return [lst[i:i + n] for i in range(0, len(lst), n)]


# -------------------------------------------------------------- graph builder
def _build(meta):
    h_pairs, w_pairs = meta['h_pairs'], meta['w_pairs']
    bh_pairs, bw_pairs = meta['bh_pairs'], meta['bw_pairs']
    PH, PW = meta['PH'], meta['PW']
    halo_h, halo_w = meta['halo_h'], meta['halo_w']
    colf, colb, col8 = meta['colf'], meta['colb'], meta['col8']

    nc = bacc.Bacc()
    x_p = nc.declare_dram_parameter("x", (C, HW), bf16, isOutput=False)
    x8_p = nc.declare_dram_parameter("x8", (C, PH * PW), f8, isOutput=False)
    cf_p = nc.declare_dram_parameter("consts", (C, meta['ckf']), f32,
                                     isOutput=False)
    cb_p = nc.declare_dram_parameter("constb", (C, meta['ckb']), bf16,
                                     isOutput=False)
    c8_p = nc.declare_dram_parameter("constf8", (C, meta['ck8']), f8,
                                     isOutput=False)
    o_p = nc.declare_dram_parameter("out", (C, HW), bf16, isOutput=True)

    with tile.TileContext(nc) as tc, ExitStack() as ctx:
        big = ctx.enter_context(tc.tile_pool(name="big", bufs=1))
        sm = ctx.enter_context(tc.tile_pool(name="sm", bufs=4))
        psq = ctx.enter_context(tc.tile_pool(name="psq", bufs=3, space="PSUM"))
        pss = ctx.enter_context(tc.tile_pool(name="pss", bufs=2, space="PSUM"))

        cst = big.tile([C, meta['ckf']], f32, tag="cst")
        cbt = big.tile([C, meta['ckb']], bf16, tag="cbt")
        c8t = big.tile([C, meta['ck8']], f8, tag="c8t")
        x_sb = big.tile([C, HW], bf16, tag="x")
        x8_sb = big.tile([C, PH * PW], f8, tag="x8")

        # --- DMA schedule -------------------------------------------------
        # x8 pieces: padded-row groups, contiguous in DRAM and SBUF.
        rpp = max(1, 1600 // PW)             # ~1.6KB pieces
        np8 = (PH + rpp - 1) // rpp
        p8 = []
        for p in range(np8):
            a = p * rpp * PW
            b = min((p + 1) * rpp, PH) * PW
            p8.append((a, b))
        # consts on the ACT queue (idle early; warmers only need cbt which
        # lands first); x chunks + early x8 pieces interleave on SP; the
        # first halo pieces and the late pieces go via the gpsimd SWDGE.
        nc.scalar.dma_start(cbt[:], cb_p[:])
        nc.scalar.dma_start(cst[:], cf_p[:])
        nc.scalar.dma_start(c8t[:], c8_p[:])
        sp_seq = [('x', 0), ('p8', 0), ('p8', 1), ('x', 1), ('p8', 4),
                  ('x', 2), ('p8', 5), ('x', 3), ('p8', 6), ('x', 4),
                  ('p8', 7), ('x', 5)] + [('x', j) for j in range(6, NBLK)]
        for kind, idx in sp_seq:
            if kind == 'x':
                sl = slice(idx * HW // NBLK, (idx + 1) * HW // NBLK)
                nc.sync.dma_start(x_sb[:, sl], x_p[:, sl])
            elif idx < np8:
                a, b = p8[idx]
                nc.sync.dma_start(x8_sb[:, a:b], x8_p[:, a:b])
        for p in list(range(2, 4)) + list(range(8, np8)):
            a, b = p8[p]
            nc.gpsimd.dma_start(x8_sb[:, a:b], x8_p[:, a:b])

        def cc(name, rows=C):
            p0, n = colf[name]
            return cst[0:rows, p0:p0 + 1]

        def cb_(name, rows=C):
            p0, n = colb[name]
            return cbt[0:rows, p0:p0 + n]

        def c8_(name):
            p0, n = col8[name]
            return c8t[:, p0:p0 + n].rearrange("p (two m) -> p two m", two=2)

        zcol = cc('zero')

        # --- warmup: ACT tables once; PE p-state ramp during fill ---------
        wrm = sm.tile([C, 4], f32, tag="wrm")
        nc.scalar.activation(wrm[:, 0:1], zcol, AF.Prelu, bias=zcol,
                             scale=1.0, alpha=cc('act_a'))
        nc.scalar.activation(wrm[:, 1:2], zcol, AF.Sigmoid, bias=zcol,
                             scale=1.0)
        psw = pss.tile([C, 512], f32, tag="small")
        for wi in range(3):
            nc.tensor.matmul(psw[:], cbt[0:C, 0:C], cbt[0:C, 0:512],
                             start=(wi == 0), stop=(wi == 2))

        ac_sb = big.tile([C, HW], bf16, tag="ac")
        ac3 = ac_sb[:].rearrange("p (h w) -> p h w", w=W)
        x8b = x8_sb[:]

        xw4 = big.tile([C, 4 * W], fp16, tag="xw4")
        nc.vector.memset(xw4[:], 0)
        yin = big.tile([C, H], bf16, tag="yin")
        yinw = big.tile([C, W], bf16, tag="yinw")
        ah = big.tile([C, H], bf16, tag="ah")

        CH = 4              # psum chunk rows (512 f32 cols = one PSUM bank)

        def pair_rhs(r0, pair, axis):
            """Manual AP: (part, 2, CH rows, W cols) windows of padded x8."""
            d1, d2 = pair
            if d2 is None:
                d2 = d1
            if axis == 'h':
                o1 = (halo_h + r0 + d1) * PW + halo_w
                o2 = (halo_h + r0 + d2) * PW + halo_w
            else:
                o1 = (halo_h + r0) * PW + halo_w + d1
                o2 = (halo_h + r0) * PW + halo_w + d2
            ap_list = [list(x8b.ap[0]), [int(o2 - o1), 2], [PW, CH], [1, W]]
            return bass.AP(tensor=x8b.tensor, offset=x8b.offset + o1,
                           ap=ap_list)

        mms = ([(f'AH{i}', p, 'h') for i, p in enumerate(h_pairs)]
               + [(f'AW{i}', p, 'w') for i, p in enumerate(w_pairs)]
               + [('BH0', bh_pairs[0], 'h'), ('BW0', bw_pairs[0], 'w')])

        # --- main pipeline: 8-row blocks (two 4-row psum groups sharing a
        # 2-bank psum tile, single merged evict); the last block is split
        # into two 4-row units so the aw-chain dependency drains faster.
        # The per-block coord-att (ah) chain is emitted one block late so
        # the tiny PE matmuls never stall the in-order PE queue.
        bounds = ([(j * BH, (j + 1) * BH) for j in range(NBLK - 1)]
                  + [(H - BH, H - CH), (H - CH, H)])

        def ah_chain(r0, r1):
            bh = r1 - r0
            pblk = pss.tile([C, 16], f32, tag="small")
            y1b = pblk[0:MIP, 0:bh]
            nc.tensor.matmul(y1b, cb_('caw1_t'), yin[:, r0:r1],
                             start=True, stop=True)
            y2b = sm.tile([MIP, BH], bf16, tag="y2b")
            nc.scalar.activation(y2b[:, 0:bh], y1b, AF.Prelu,
                                 bias=cc('cab', rows=MIP),
                                 scale=cc('cas', rows=MIP),
                                 alpha=cc('caa', rows=MIP))
            ahp = pblk[:, 8:8 + bh]
            nc.tensor.matmul(ahp, cb_('cawh_t', rows=MIP), y2b[:, 0:bh],
                             start=True, stop=True)
            nc.scalar.activation(ah[:, r0:r1], ahp, AF.Sigmoid,
                                 bias=zcol, scale=1.0)
            # apply ah in place on the idle gpsimd engine
            ah_b = ah[:, r0:r1].unsqueeze(2).broadcast_to((C, bh, W))
            nc.gpsimd.tensor_tensor(ac3[:, r0:r1, :],
                                    ac3[:, r0:r1, :], ah_b, op=ALU.mult)

        prev = None
        for r0, r1 in bounds:
            bh = r1 - r0
            sl = slice(r0 * W, r1 * W)
            pk = psq.tile([C, bh, W], f32, tag="pk")
            for k in range(bh // CH):
                cr0 = r0 + k * CH
                pkk = pk[:, k * CH:(k + 1) * CH, :]
                nc.tensor.matmul(pkk, cb_('Wp'),
                                 x_sb[:, cr0 * W:(cr0 + CH) * W],
                                 start=True, stop=False)
                for mi, (nm, pair, axis) in enumerate(mms):
                    nc.tensor.matmul(pkk, c8_(nm),
                                     pair_rhs(cr0, pair, axis),
                                     start=False, stop=(mi == len(mms) - 1),
                                     perf_mode=PM.DoubleRow)
            # evict with bias + PReLU (whole block, one ACT op)
            nc.scalar.activation(ac_sb[:, sl], pk[:], AF.Prelu,
                                 bias=cc('bz'), scale=1.0, alpha=cc('act_a'))
            # xw column sums (fp16 accumulator, mod-4 rows)
            for k in range(bh // CH):
                cs_ = (r0 + k * CH) * W
                nc.vector.tensor_tensor(xw4[:], xw4[:],
                                        ac_sb[:, cs_:cs_ + CH * W],
                                        op=ALU.add)
            # xh row sums: fold halves then reduce
            fold = sm.tile([C, BH * (W // 2)], bf16, tag="fold")
            f3 = fold[:].rearrange("p (h w) -> p h w", w=W // 2)[:, 0:bh, :]
            nc.vector.tensor_tensor(f3, ac3[:, r0:r1, 0:W // 2],
                                    ac3[:, r0:r1, W // 2:W], op=ALU.add)
            with nc.allow_low_precision(reason="xh sums feed smooth gates"):
                nc.vector.tensor_reduce(yin[:, r0:r1], f3,
                                        axis=mybir.AxisListType.X, op=ALU.add)
            if prev is not None:
                ah_chain(*prev)
            prev = (r0, r1)
        ah_chain(*prev)

        # --- tail: aw gate (xw reduced via two folds, 2x DVE mode) --------
        xwA = sm.tile([C, 2 * W], fp16, tag="xwA")
        nc.vector.tensor_tensor(xwA[:], xw4[:, 0:2 * W], xw4[:, 2 * W:4 * W],
                                op=ALU.add)
        nc.vector.tensor_tensor(yinw[:], xwA[:, 0:W], xwA[:, W:2 * W],
                                op=ALU.add)
        y1w = pss.tile([MIP, W], f32, tag="small")
        nc.tensor.matmul(y1w[:], cb_('caw1_t'), yinw[:], start=True, stop=True)
        y2w = sm.tile([MIP, W], bf16, tag="y2w")
        nc.scalar.activation(y2w[:], y1w[:], AF.Prelu,
                             bias=cc('cab', rows=MIP),
                             scale=cc('cas', rows=MIP),
                             alpha=cc('caa', rows=MIP))
        awp = pss.tile([C, W], f32, tag="small")
        nc.tensor.matmul(awp[:], cb_('caww_t', rows=MIP), y2w[:],
                         start=True, stop=True)
        aw = sm.tile([C, W], bf16, tag="aw")
        nc.scalar.activation(aw[:], awp[:], AF.Sigmoid, bias=zcol, scale=1.0)

        aw_b = aw[:].unsqueeze(1).broadcast_to((C, BH, W))
        for j in range(NBLK):
            r0 = j * BH
            eng = nc.gpsimd if (j % 3 == 1 or j == 15) else nc.vector
            eng.tensor_tensor(ac3[:, r0:r0 + BH, :], ac3[:, r0:r0 + BH, :],
                              aw_b, op=ALU.mult)
            q = nc.gpsimd if j == 14 else (nc.sync if j % 2 == 0
                                           else nc.scalar)
            q.dma_start(o_p[:, r0 * W:(r0 + BH) * W],
                        ac_sb[:, r0 * W:(r0 + BH) * W])

    nc.compile()
    return nc


def _get_graph(meta, key):
    if key not in _GRAPH_CACHE:
        _GRAPH_CACHE[key] = _build(meta)
    return _GRAPH_CACHE[key]


# ------------------------------------------------------------------ interface
def _run(inputs, trace=False):
    x = np.ascontiguousarray(np.asarray(inputs['x'], np.float32))
    assert x.shape == (B, C, H, W)
    consts, constb, constf8, meta, key = _fold(inputs)
    nc = _get_graph(meta, key)
    PH, PW = meta['PH'], meta['PW']
    hh, hw = meta['halo_h'], meta['halo_w']
    xb = x.astype(ml_dtypes.bfloat16)
    xs = (x / meta['s8'][None, :, None, None].astype(np.float32))
    xpad = np.zeros((B, C, PH, PW), ml_dtypes.float8_e4m3)
    xpad[:, :, hh:hh + H, hw:hw + W] = xs.astype(ml_dtypes.float8_e4m3)
    in_maps = []
    for i in range(N_CORES):
        in_maps.append({'x': xb[i].reshape(C, HW).copy(),
                        'x8': xpad[i].reshape(C, PH * PW).copy(),
                        'consts': consts, 'constb': constb,
                        'constf8': constf8})
    res = run_bass_kernel_spmd(nc, in_maps, list(range(N_CORES)), trace=trace)
    out = np.stack([res.results[i]['out'].astype(np.float32).reshape(C, H, W)
                    for i in range(N_CORES)], axis=0)
    return out, res


def kernel(**inputs):
    out, _ = _run(inputs, trace=False)
    return out
